# revision 32
# baseline (speedup 1.0000x reference)
"""CTC loss (K.ctc_batch_cost with full lengths, blank=C-1) on 8 Trainium2 cores.

Pure data parallelism: 128 batch rows per core, batch on SBUF partitions.

Per core:
  1. Stream y_pred [128,256,100] into SBUF in a "group" layout (16 tiles; tile n
     holds batches 8n..8n+8, one batch per 16-partition group, 16 time rows per
     partition), ap_gather the 64 label classes + blank per time row (per-group
     index streams precomputed on host from y_true), and DMA-shuffle each
     result to batch-major pl_bm[b, t*65+l] (l=64 is the blank).
  2. Prescale: per-batch F = exp(C0 - mean_t ln max_l p) keeps probability-space
     alphas inside fp32 range for the whole sequence, then
     pl_bm <- (pl_bm + EPS) * F.
  3. CTC forward recursion in probability space, processed lattice-ROW-wise:
     each of the 129 extended-label states is one first-order recurrence along
     time, computed by a single tensor_tensor_scan instruction
         alpha_t[s] = p_t[s]*alpha_{t-1}[s] + p_t[s]*c_t[s],
         c_t[s] = alpha_{t-1}[s-1] + allow[s]*alpha_{t-1}[s-2]
     (2-3 instructions per row instead of 5 instructions per time step; this
     environment's cost is dominated by instruction-dependency latency).
  4. loss = 256*ln(F) - ln(alpha_T[S-1] + alpha_T[S-2]).
"""
import numpy as np

B, T, C, L = 1024, 256, 100, 64
BLOC = 128               # batches per core
S1 = L + 1               # 65 gather slots per time row (64 labels + blank)
S = 2 * L + 1            # 129 lattice rows
NIDX = 16 * S1           # 1040 idxs per gather instruction
IDXCOLS = NIDX // 16     # 65 idx columns actually read per tile
IDXPITCH = 66            # storage pitch (even -> every tile slice 4B-aligned)
ROWELEMS = 16 * C        # 1600 y_pred elems per partition per tile
EPS = 1e-7
C0 = 1.04                # calibration of the greedy-decay proxy (nats/step)

_compiled = None


def make_idx(y_true_loc: np.ndarray) -> np.ndarray:
    """Wrapped ap_gather index streams [128, 16*65] int16 for one core.

    Gather tile n covers batches 8n..8n+8 (one per 16-partition group g); the
    partition 16g+r holds time rows 16r..16r+16. Stream element j = k*65 + l
    has idx value k*100 + (label l | blank), stored wrapped at
    [16g + j%16, n*65 + j//16].
    """
    idx = np.zeros((128, 16 * IDXPITCH), np.int16)
    j = np.arange(NIDX)
    k, l = j // S1, j % S1
    lab = np.minimum(l, L - 1)
    part = j % 16
    col = j // 16
    for b in range(BLOC):
        n, g = b // 8, b % 8
        vals = (k * C + np.where(l < L, y_true_loc[b, lab], C - 1)).astype(np.int16)
        idx[16 * g + part, n * IDXPITCH + col] = vals
    return idx


def build(nc, repeats: int = 1):
    import concourse.mybir as mybir
    from concourse import tile

    f32 = mybir.dt.float32
    Alu = mybir.AluOpType
    Act = mybir.ActivationFunctionType

    yp = nc.dram_tensor("yp", [BLOC, T, C], f32, kind="ExternalInput")
    yt = nc.dram_tensor("yt", [BLOC, L], mybir.dt.int32, kind="ExternalInput")
    idx = nc.dram_tensor("idx", [128, 16 * IDXPITCH], mybir.dt.int16,
                         kind="ExternalInput")
    loss = nc.dram_tensor("loss", [BLOC, 1], f32, kind="ExternalOutput")

    with tile.TileContext(nc) as tc:
        with (
            tc.tile_pool(name="ypg", bufs=14) as ypg_pool,
            tc.tile_pool(name="plg", bufs=6) as plg_pool,
            tc.tile_pool(name="misc", bufs=1) as misc,
        ):
            idx_sb = misc.tile([128, 16 * IDXPITCH], mybir.dt.int16)
            yt_sb = misc.tile([128, L], mybir.dt.int32)
            allow = misc.tile([128, L], f32)
            pl_bm = misc.tile([128, T * S1], f32)
            # fwd row tiles have a leading pad column (always 0) so the shifted
            # reads at t=0 are in-bounds; alpha_t lives at col t+1.
            R = [misc.tile([128, T + 1], f32, name=f"row{i}") for i in range(3)]
            # bwd tiles: bG col m = G_{255-m}[s] (col0 = terminal value);
            # bH col m = H_{255-m}[s] = p*G, m=0..254.
            bG = [misc.tile([128, T], f32, name=f"bG{i}") for i in range(3)]
            bH = [misc.tile([128, T - 1], f32, name=f"bH{i}") for i in range(3)]
            czero = misc.tile([128, T], f32)
            ctile = [misc.tile([128, T], f32, name=f"ct{i}") for i in range(2)]
            dtile = [misc.tile([128, T], f32, name=f"dt{i}") for i in range(2)]
            cbtile = [misc.tile([128, T - 1], f32, name=f"cb{i}") for i in range(2)]
            sptile = misc.tile([128, T - 1], f32)
            sztile = misc.tile([128, T - 1], f32)
            mx = misc.tile([128, T], f32)
            smx = misc.tile([128, 1], f32)
            lnF = misc.tile([128, 1], f32)
            Fb = misc.tile([128, 1], f32)
            biasEF = misc.tile([128, 1], f32)
            tot = misc.tile([128, 1], f32)
            logtot = misc.tile([128, 1], f32)
            loss_sb = misc.tile([128, 1], f32)

            nc.sync.dma_start(idx_sb[:], idx.ap())
            nc.sync.dma_start(yt_sb[:], yt.ap())

            # allow[j] = 1 if j==0 else (y[j] != y[j-1])
            nc.vector.tensor_tensor(
                allow[:, 1:L], yt_sb[:, 1:L], yt_sb[:, 0:L - 1], op=Alu.not_equal)
            nc.gpsimd.memset(allow[:, 0:1], 1.0)
            nc.gpsimd.memset(czero[:], 0.0)

            for _rep in range(repeats):
                # ---- phase 1: load + gather + shuffle to pl_bm ----
                ypr = yp.ap().rearrange("(n g) t c -> n g (t c)", g=8)
                ypgs, plgs = [], []
                for n in range(16):
                    ypg = ypg_pool.tile([128, ROWELEMS], f32,
                                        name=f"ypg_{n}", tag="ypg")
                    src = ypr[n, :, :].rearrange("g (r y) -> g r y", y=ROWELEMS)
                    nc.sync.dma_start(ypg[:], src)
                    ypgs.append(ypg)
                for n in range(16):
                    plg = plg_pool.tile([128, NIDX], f32, name=f"plg_{n}",
                                        tag="plg")
                    nc.gpsimd.ap_gather(
                        plg[:], ypgs[n][:],
                        idx_sb[:, n * IDXPITCH:n * IDXPITCH + IDXCOLS],
                        channels=128, num_elems=ROWELEMS, d=1, num_idxs=NIDX)
                    plgs.append(plg)
                for n in range(16):
                    dst = pl_bm[8 * n:8 * (n + 1), :].rearrange(
                        "p (r j) -> p r j", j=NIDX)
                    nc.scalar.dma_start(dst, plgs[n][:])

                # ---- phase 2: per-batch prescale F, pl_bm <- (pl_bm+EPS)*F ----
                pl_v = pl_bm[:].rearrange("p (t l) -> p t l", l=S1)
                nc.vector.tensor_reduce(
                    mx[:].rearrange("p t -> p t ()"), pl_v,
                    axis=mybir.AxisListType.X, op=Alu.max)
                nc.scalar.activation(mx[:], mx[:], Act.Ln)
                nc.vector.tensor_reduce(
                    smx[:], mx[:], axis=mybir.AxisListType.X, op=Alu.add)
                nc.vector.tensor_scalar(
                    lnF[:], smx[:], -1.0 / T, C0, op0=Alu.mult, op1=Alu.add)
                nc.scalar.activation(Fb[:], lnF[:], Act.Exp)
                nc.vector.tensor_scalar(
                    biasEF[:], Fb[:], EPS, None, op0=Alu.mult)
                nc.vector.tensor_scalar(
                    pl_bm[:], pl_bm[:], Fb[:], biasEF[:],
                    op0=Alu.mult, op1=Alu.add)

                # ---- phase 3: lattice rows as scans along t, computed from
                # BOTH ends of the lattice at once (two independent dependency
                # chains that pipeline against each other).
                # Forward rows 0..64: alpha_t[s] = p_t[s]*(alpha_{t-1}[s]+c_t[s]),
                # scan covers t=0..255 with the t=0 term riding on the initial
                # (=1 for s<2 else 0) and c_0 = 0 via the pad column.
                # Backward rows 128..65: G_t[s] = p_{t+1}[s]*G_{t+1}[s] + cB,
                # cB_t[s] = H_{t+1}[s+1] + a[s+2]*H_{t+1}[s+2], H = p*G,
                # scanned in reversed time (col m = time 255-m).
                if _rep == 0:
                    for i in range(3):
                        nc.gpsimd.memset(R[i][:, 0:1], 0.0)

                pl_view = pl_bm[:].rearrange("p (t l) -> p t l", l=S1)

                def emit_fwd(s):
                    j = s // 2
                    col = j if s % 2 == 1 else L
                    p_sl = pl_view[:, 0:T, col]          # [128, 256]
                    rt = R[s % 3]
                    dt = dtile[(s // 2) % 2]
                    ct = ctile[(s // 2) % 2]
                    if s == 0:
                        d = czero[:]
                    elif s % 2 == 0 or s == 1:
                        nc.vector.tensor_tensor(
                            dt[:], R[(s - 1) % 3][:, 0:T], p_sl, op=Alu.mult)
                        d = dt[:]
                    else:
                        nc.vector.scalar_tensor_tensor(
                            ct[:], R[(s - 2) % 3][:, 0:T],
                            allow[:, j:j + 1], R[(s - 1) % 3][:, 0:T],
                            op0=Alu.mult, op1=Alu.add)
                        nc.vector.tensor_tensor(dt[:], ct[:], p_sl,
                                                op=Alu.mult)
                        d = dt[:]
                    nc.vector.tensor_tensor_scan(
                        rt[:, 1:T + 1], p_sl, d, 1.0 if s < 2 else 0.0,
                        op0=Alu.mult, op1=Alu.add)

                def emit_bwd(s):
                    j = s // 2
                    col = j if s % 2 == 1 else L
                    p_rev = pl_view[:, T - 1:0:-1, col]  # [128, 255] t=255..1
                    gt = bG[s % 3]
                    # col0 = terminal G_255[s]: 1 for s>=127, else 0
                    if s >= 127:
                        nc.gpsimd.memset(gt[:, 0:1], 1.0)
                    elif s >= 124:
                        # first reuse of each rotating tile: clear terminal col
                        nc.gpsimd.memset(gt[:, 0:1], 0.0)
                    if s == 128:
                        d = czero[:, 0:T - 1]
                    elif s % 2 == 0 or s == 127:
                        # even s: a[s+2]=0 (s+2 even); s=127: s+2 out of range
                        d = bH[(s + 1) % 3][:]
                    else:
                        cbt = cbtile[(s // 2) % 2]
                        nc.vector.scalar_tensor_tensor(
                            cbt[:], bH[(s + 2) % 3][:],
                            allow[:, (s + 2) // 2:(s + 2) // 2 + 1],
                            bH[(s + 1) % 3][:], op0=Alu.mult, op1=Alu.add)
                        d = cbt[:]
                    nc.vector.tensor_tensor_scan(
                        gt[:, 1:T], p_rev, d, 1.0 if s >= 127 else 0.0,
                        op0=Alu.mult, op1=Alu.add)
                    nc.vector.tensor_tensor(
                        bH[s % 3][:], gt[:, 0:T - 1], p_rev, op=Alu.mult)

                for i in range(65):
                    emit_fwd(i)
                    if i < 64:
                        emit_bwd(128 - i)

                # ---- splice across the 64|65 boundary ----
                # P = sum_t (alpha_t[64] + a[65]*alpha_t[63]) * H_{t+1}[65]
                nc.vector.scalar_tensor_tensor(
                    sptile[:], R[63 % 3][:, 1:T], allow[:, 32:33],
                    R[64 % 3][:, 1:T], op0=Alu.mult, op1=Alu.add)
                nc.vector.tensor_tensor(
                    sztile[:], sptile[:], bH[65 % 3][:, T - 2::-1], op=Alu.mult)
                nc.vector.tensor_reduce(
                    tot[:], sztile[:], axis=mybir.AxisListType.X, op=Alu.add)
                nc.scalar.activation(logtot[:], tot[:], Act.Ln)
                nc.vector.scalar_tensor_tensor(
                    loss_sb[:], lnF[:], float(T), logtot[:],
                    op0=Alu.mult, op1=Alu.subtract)
            nc.sync.dma_start(loss.ap(), loss_sb[:])
    nc.compile()
    return nc


def _get_compiled():
    global _compiled
    if _compiled is None:
        import concourse.bacc as bacc
        nc = bacc.Bacc("TRN2", target_bir_lowering=False, debug=False, num_devices=1)
        _compiled = build(nc)
    return _compiled


def kernel(y_true: np.ndarray, y_pred: np.ndarray) -> np.ndarray:
    from concourse.bass_utils import run_bass_kernel_spmd

    nc = _get_compiled()
    y_true = np.asarray(y_true)
    y_pred = np.asarray(y_pred, dtype=np.float32)
    in_maps = []
    for c in range(8):
        sl = slice(c * BLOC, (c + 1) * BLOC)
        ytc = np.ascontiguousarray(y_true[sl]).astype(np.int32, copy=False)
        in_maps.append({
            "yp": np.ascontiguousarray(y_pred[sl]),
            "yt": ytc,
            "idx": make_idx(ytc),
        })
    res = run_bass_kernel_spmd(nc, in_maps, core_ids=list(range(8)))
    return np.concatenate([res.results[c]["loss"] for c in range(8)], axis=0)


# revision 33
# speedup vs baseline: 3.4854x; 3.4854x over previous
"""CTC loss (K.ctc_batch_cost with full lengths, blank=C-1) on 8 Trainium2 cores.

Pure data parallelism: 128 batch rows per core, batch on SBUF partitions.

Per core:
  1. Stream y_pred [128,256,100] into SBUF in a "group" layout (16 tiles; tile n
     holds batches 8n..8n+8, one batch per 16-partition group, 16 time rows per
     partition), ap_gather the 64 label classes + blank per time row (per-group
     index streams precomputed on host from y_true), and DMA-shuffle each
     result to batch-major pl_bm[b, t*65+l] (l=64 is the blank).
  2. Prescale: per-batch F = exp(C0 - mean_t ln max_l p) keeps probability-space
     alphas inside fp32 range for the whole sequence, then
     pl_bm <- (pl_bm + EPS) * F.
  3. CTC forward recursion in probability space, processed lattice-ROW-wise:
     each of the 129 extended-label states is one first-order recurrence along
     time, computed by a single tensor_tensor_scan instruction
         alpha_t[s] = p_t[s]*alpha_{t-1}[s] + p_t[s]*c_t[s],
         c_t[s] = alpha_{t-1}[s-1] + allow[s]*alpha_{t-1}[s-2]
     (2-3 instructions per row instead of 5 instructions per time step; this
     environment's cost is dominated by instruction-dependency latency).
  4. loss = 256*ln(F) - ln(alpha_T[S-1] + alpha_T[S-2]).
"""
import numpy as np

B, T, C, L = 1024, 256, 100, 64
BLOC = 128               # batches per core
S1 = L + 1               # 65 gather slots per time row (64 labels + blank)
S = 2 * L + 1            # 129 lattice rows
NIDX = 16 * S1           # 1040 idxs per gather instruction
IDXCOLS = NIDX // 16     # 65 idx columns actually read per tile
IDXPITCH = 66            # storage pitch (even -> every tile slice 4B-aligned)
ROWELEMS = 16 * C        # 1600 y_pred elems per partition per tile
EPS = 1e-7
C0 = 1.04                # calibration of the greedy-decay proxy (nats/step)

_compiled = None


def make_idx(y_true_loc: np.ndarray) -> np.ndarray:
    """Wrapped ap_gather index streams [128, 16*65] int16 for one core.

    Gather tile n covers batches 8n..8n+8 (one per 16-partition group g); the
    partition 16g+r holds time rows 16r..16r+16. Stream element j = k*65 + l
    has idx value k*100 + (label l | blank), stored wrapped at
    [16g + j%16, n*65 + j//16].
    """
    idx = np.zeros((128, 16 * IDXPITCH), np.int16)
    j = np.arange(NIDX)
    k, l = j // S1, j % S1
    lab = np.minimum(l, L - 1)
    part = j % 16
    col = j // 16
    for b in range(BLOC):
        n, g = b // 8, b % 8
        vals = (k * C + np.where(l < L, y_true_loc[b, lab], C - 1)).astype(np.int16)
        idx[16 * g + part, n * IDXPITCH + col] = vals
    return idx


def build(nc, repeats: int = 1):
    import concourse.mybir as mybir
    from concourse import tile

    f32 = mybir.dt.float32
    Alu = mybir.AluOpType
    Act = mybir.ActivationFunctionType

    yp = nc.dram_tensor("yp", [BLOC, T, C], f32, kind="ExternalInput")
    yt = nc.dram_tensor("yt", [BLOC, L], mybir.dt.int32, kind="ExternalInput")
    idx = nc.dram_tensor("idx", [128, 16 * IDXPITCH], mybir.dt.int16,
                         kind="ExternalInput")
    loss = nc.dram_tensor("loss", [BLOC, 1], f32, kind="ExternalOutput")

    with tile.TileContext(nc) as tc:
        with (
            tc.tile_pool(name="ypg", bufs=14) as ypg_pool,
            tc.tile_pool(name="plg", bufs=6) as plg_pool,
            tc.tile_pool(name="misc", bufs=1) as misc,
        ):
            idx_sb = misc.tile([128, 16 * IDXPITCH], mybir.dt.int16)
            yt_sb = misc.tile([128, L], mybir.dt.int32)
            allow = misc.tile([128, L], f32)
            pl_bm = misc.tile([128, T * S1], f32)
            # fwd row tiles have a leading pad column (always 0) so the shifted
            # reads at t=0 are in-bounds; alpha_t lives at col t+1.
            R = [misc.tile([128, T + 1], f32, name=f"row{i}") for i in range(3)]
            # bwd tiles: bG col m = G_{255-m}[s] (col0 = terminal value);
            # bH col m = H_{255-m}[s] = p*G, m=0..254.
            bG = [misc.tile([128, T], f32, name=f"bG{i}") for i in range(3)]
            bH = [misc.tile([128, T - 1], f32, name=f"bH{i}") for i in range(3)]
            czero = misc.tile([128, T], f32)
            ctile = [misc.tile([128, T], f32, name=f"ct{i}") for i in range(2)]
            dtile = [misc.tile([128, T], f32, name=f"dt{i}") for i in range(2)]
            cbtile = [misc.tile([128, T - 1], f32, name=f"cb{i}") for i in range(2)]
            sptile = misc.tile([128, T - 1], f32)
            sztile = misc.tile([128, T - 1], f32)
            mx = misc.tile([128, T], f32)
            smx = misc.tile([128, 1], f32)
            lnF = misc.tile([128, 1], f32)
            Fb = misc.tile([128, 1], f32)
            biasEF = misc.tile([128, 1], f32)
            tot = misc.tile([128, 1], f32)
            logtot = misc.tile([128, 1], f32)
            loss_sb = misc.tile([128, 1], f32)

            nc.sync.dma_start(idx_sb[:], idx.ap())
            nc.sync.dma_start(yt_sb[:], yt.ap())

            # allow[j] = 1 if j==0 else (y[j] != y[j-1])
            nc.vector.tensor_tensor(
                allow[:, 1:L], yt_sb[:, 1:L], yt_sb[:, 0:L - 1], op=Alu.not_equal)
            nc.vector.memset(allow[:, 0:1], 1.0)
            nc.vector.memset(czero[:], 0.0)

            for _rep in range(repeats):
                # ---- phase 1: load + gather + shuffle to pl_bm ----
                ypr = yp.ap().rearrange("(n g) t c -> n g (t c)", g=8)
                ypgs, plgs = [], []
                for n in range(16):
                    ypg = ypg_pool.tile([128, ROWELEMS], f32,
                                        name=f"ypg_{n}", tag="ypg")
                    src = ypr[n, :, :].rearrange("g (r y) -> g r y", y=ROWELEMS)
                    nc.sync.dma_start(ypg[:], src)
                    ypgs.append(ypg)
                for n in range(16):
                    plg = plg_pool.tile([128, NIDX], f32, name=f"plg_{n}",
                                        tag="plg")
                    nc.gpsimd.ap_gather(
                        plg[:], ypgs[n][:],
                        idx_sb[:, n * IDXPITCH:n * IDXPITCH + IDXCOLS],
                        channels=128, num_elems=ROWELEMS, d=1, num_idxs=NIDX)
                    plgs.append(plg)
                for n in range(16):
                    dst = pl_bm[8 * n:8 * (n + 1), :].rearrange(
                        "p (r j) -> p r j", j=NIDX)
                    nc.scalar.dma_start(dst, plgs[n][:])

                # ---- phase 2: per-batch prescale F, pl_bm <- (pl_bm+EPS)*F ----
                pl_v = pl_bm[:].rearrange("p (t l) -> p t l", l=S1)
                nc.vector.tensor_reduce(
                    mx[:].rearrange("p t -> p t ()"), pl_v,
                    axis=mybir.AxisListType.X, op=Alu.max)
                nc.scalar.activation(mx[:], mx[:], Act.Ln)
                nc.vector.tensor_reduce(
                    smx[:], mx[:], axis=mybir.AxisListType.X, op=Alu.add)
                nc.vector.tensor_scalar(
                    lnF[:], smx[:], -1.0 / T, C0, op0=Alu.mult, op1=Alu.add)
                nc.scalar.activation(Fb[:], lnF[:], Act.Exp)
                nc.vector.tensor_scalar(
                    biasEF[:], Fb[:], EPS, None, op0=Alu.mult)
                nc.vector.tensor_scalar(
                    pl_bm[:], pl_bm[:], Fb[:], biasEF[:],
                    op0=Alu.mult, op1=Alu.add)

                # ---- phase 3: lattice rows as scans along t, computed from
                # BOTH ends of the lattice at once (two independent dependency
                # chains that pipeline against each other).
                # Forward rows 0..64: alpha_t[s] = p_t[s]*(alpha_{t-1}[s]+c_t[s]),
                # scan covers t=0..255 with the t=0 term riding on the initial
                # (=1 for s<2 else 0) and c_0 = 0 via the pad column.
                # Backward rows 128..65: G_t[s] = p_{t+1}[s]*G_{t+1}[s] + cB,
                # cB_t[s] = H_{t+1}[s+1] + a[s+2]*H_{t+1}[s+2], H = p*G,
                # scanned in reversed time (col m = time 255-m).
                if _rep == 0:
                    for i in range(3):
                        nc.vector.memset(R[i][:, 0:1], 0.0)

                pl_view = pl_bm[:].rearrange("p (t l) -> p t l", l=S1)

                def emit_fwd(s):
                    j = s // 2
                    col = j if s % 2 == 1 else L
                    p_sl = pl_view[:, 0:T, col]          # [128, 256]
                    rt = R[s % 3]
                    dt = dtile[(s // 2) % 2]
                    ct = ctile[(s // 2) % 2]
                    if s == 0:
                        d = czero[:]
                    elif s % 2 == 0 or s == 1:
                        nc.vector.tensor_tensor(
                            dt[:], R[(s - 1) % 3][:, 0:T], p_sl, op=Alu.mult)
                        d = dt[:]
                    else:
                        nc.vector.scalar_tensor_tensor(
                            ct[:], R[(s - 2) % 3][:, 0:T],
                            allow[:, j:j + 1], R[(s - 1) % 3][:, 0:T],
                            op0=Alu.mult, op1=Alu.add)
                        nc.vector.tensor_tensor(dt[:], ct[:], p_sl,
                                                op=Alu.mult)
                        d = dt[:]
                    nc.vector.tensor_tensor_scan(
                        rt[:, 1:T + 1], p_sl, d, 1.0 if s < 2 else 0.0,
                        op0=Alu.mult, op1=Alu.add)

                def emit_bwd(s):
                    j = s // 2
                    col = j if s % 2 == 1 else L
                    p_rev = pl_view[:, T - 1:0:-1, col]  # [128, 255] t=255..1
                    gt = bG[s % 3]
                    # col0 = terminal G_255[s]: 1 for s>=127, else 0
                    if s >= 127:
                        nc.vector.memset(gt[:, 0:1], 1.0)
                    elif s >= 124:
                        # first reuse of each rotating tile: clear terminal col
                        nc.vector.memset(gt[:, 0:1], 0.0)
                    if s == 128:
                        d = czero[:, 0:T - 1]
                    elif s % 2 == 0 or s == 127:
                        # even s: a[s+2]=0 (s+2 even); s=127: s+2 out of range
                        d = bH[(s + 1) % 3][:]
                    else:
                        cbt = cbtile[(s // 2) % 2]
                        nc.vector.scalar_tensor_tensor(
                            cbt[:], bH[(s + 2) % 3][:],
                            allow[:, (s + 2) // 2:(s + 2) // 2 + 1],
                            bH[(s + 1) % 3][:], op0=Alu.mult, op1=Alu.add)
                        d = cbt[:]
                    nc.vector.tensor_tensor_scan(
                        gt[:, 1:T], p_rev, d, 1.0 if s >= 127 else 0.0,
                        op0=Alu.mult, op1=Alu.add)
                    nc.vector.tensor_tensor(
                        bH[s % 3][:], gt[:, 0:T - 1], p_rev, op=Alu.mult)

                for i in range(65):
                    emit_fwd(i)
                    if i < 64:
                        emit_bwd(128 - i)

                # ---- splice across the 64|65 boundary ----
                # P = sum_t (alpha_t[64] + a[65]*alpha_t[63]) * H_{t+1}[65]
                nc.vector.scalar_tensor_tensor(
                    sptile[:], R[63 % 3][:, 1:T], allow[:, 32:33],
                    R[64 % 3][:, 1:T], op0=Alu.mult, op1=Alu.add)
                nc.vector.tensor_tensor(
                    sztile[:], sptile[:], bH[65 % 3][:, T - 2::-1], op=Alu.mult)
                nc.vector.tensor_reduce(
                    tot[:], sztile[:], axis=mybir.AxisListType.X, op=Alu.add)
                nc.scalar.activation(logtot[:], tot[:], Act.Ln)
                nc.vector.scalar_tensor_tensor(
                    loss_sb[:], lnF[:], float(T), logtot[:],
                    op0=Alu.mult, op1=Alu.subtract)
            nc.sync.dma_start(loss.ap(), loss_sb[:])
    nc.compile()
    return nc


def _get_compiled():
    global _compiled
    if _compiled is None:
        import concourse.bacc as bacc
        nc = bacc.Bacc("TRN2", target_bir_lowering=False, debug=False, num_devices=1)
        _compiled = build(nc)
    return _compiled


def kernel(y_true: np.ndarray, y_pred: np.ndarray) -> np.ndarray:
    from concourse.bass_utils import run_bass_kernel_spmd

    nc = _get_compiled()
    y_true = np.asarray(y_true)
    y_pred = np.asarray(y_pred, dtype=np.float32)
    in_maps = []
    for c in range(8):
        sl = slice(c * BLOC, (c + 1) * BLOC)
        ytc = np.ascontiguousarray(y_true[sl]).astype(np.int32, copy=False)
        in_maps.append({
            "yp": np.ascontiguousarray(y_pred[sl]),
            "yt": ytc,
            "idx": make_idx(ytc),
        })
    res = run_bass_kernel_spmd(nc, in_maps, core_ids=list(range(8)))
    return np.concatenate([res.results[c]["loss"] for c in range(8)], axis=0)
